# revision 6
# baseline (speedup 1.0000x reference)
"""DeformationLoss kernel for 8 Trainium2 NeuronCores.

Math: loss = (1/num_pairs) * sum_{i<j} mean_k || d_i,k - d_j,k ||_2,
with d = pred - recon, B=512, J=32.

Strategy: shard the 32 joints across 8 cores (4 joints each); every core
computes the upper-triangle (block granularity 128) of the 512x512
pairwise squared distances for its joints via bf16 matmuls whose 8-row
operand stacks fold the full  dist^2 = n_i + n_j - 2 d_i.d_j + EPS  form:

    L = [-2dx,-2dy,-2dz, m1, m2, -1, -1, -EPS]   (lhsT rows)
    R = [ dx,  dy,  dz, -1, -1, m1, m2,  -1 ]    (rhs rows)

with m1+m2 a 2-way bf16 split of m = -n (n = ||d_i||^2 of the
bf16-rounded d), so PSUM holds dist^2 + EPS >= EPS directly and ScalarE
does deform = sqrt(G) with a fused per-partition running sum (accum_out).

Diagonal 128x128 blocks are computed at full scale; the two diagonal
slabs of a joint pair are merged into one via a triangular-mask
copy_predicated (upper half from the even joint, strict lower half from
the odd joint), so each in-block pair is counted exactly once and ACT
processes 768 / 1280 cols for even / odd joints (4096 per core vs 5120
unmerged).  Host sums the 8x[128,4] partials in f64 (no doubling).
"""

import numpy as np

B, J, C = 512, 32, 3
NCORES = 8
J_LOC = J // NCORES  # joints per core
NUM_PAIRS = B * (B - 1) // 2
EPS = 1.5e-3

_STATE = {}


def _ensure_path():
    import sys
    try:
        import concourse.bass  # noqa: F401
    except ImportError:
        for p in ("/opt/trn_rl_repo", "/root/.axon_site/_ro/trn_rl_repo"):
            if p not in sys.path:
                sys.path.insert(0, p)


def _split_multi_waits_json(bir_json: bytes) -> bytes:
    """The walrus in this image rejects instructions carrying >1 sync wait
    ("Too many sync wait commands", CoreV3GenImpl setupSyncWait).  Tile's
    scheduler emits such instructions (notably the kernel-tail Drain).
    Rewrite the BIR: hoist all-but-the-last wait of each instruction into
    dedicated single-wait NoOps right before it on the same engine."""
    import orjson

    d = orjson.loads(bir_json)
    changed = False
    for fn in d.get("functions", []):
        for bb in fn.get("blocks", []):
            out = []
            for ins in bb.get("instructions", []):
                si = ins.get("sync_info")
                waits = (si or {}).get("on_wait") or []
                if len(waits) > 1:
                    changed = True
                    for i, w in enumerate(waits[:-1]):
                        out.append({
                            "debug": ins.get("debug", 0),
                            "engine": ins["engine"],
                            "ins": [],
                            "name": f"{ins['name']}-hw{i}",
                            "opcode": "NoOp",
                            "outs": [],
                            "sync_info": {"on_update": [], "on_wait": [w]},
                        })
                    si["on_wait"] = [waits[-1]]
                out.append(ins)
            bb["instructions"] = out
    if not changed:
        return bir_json
    return orjson.dumps(d)


def install_walrus_wait_split(max_sem_num: int | None = 176):
    """Monkeypatch compile_bir_kernel so every bass compile in this process
    goes through the multi-wait splitter; optionally cap walrus's semaphore
    space so its NEFF epilogue clears fewer semaphores (the stock epilogue
    zeroes all 253 one-by-one, ~6.5us of tail)."""
    _ensure_path()
    import concourse.bass_utils as bu
    import concourse.bass2jax as b2j

    if getattr(bu, "_wait_split_installed", False):
        return
    orig = bu.compile_bir_kernel

    def patched(bir_json, tmpdir, neff_name="file.neff"):
        return orig(_split_multi_waits_json(bytes(bir_json)), tmpdir, neff_name)

    bu.compile_bir_kernel = patched
    b2j.compile_bir_kernel = patched

    if max_sem_num is not None:
        orig_args = bu.get_walrus_args

        def patched_args(*a, **k):
            return orig_args(*a, **k) + [f"--max-sem-num={max_sem_num}"]

        bu.get_walrus_args = patched_args
    bu._wait_split_installed = True


def _install_cheap_tile_teardown():
    """Replace TileContext's expensive tail (drain + all-engine barrier +
    sem clears + barrier, ~3us) with nothing.  Safe here because the NEFF
    epilogue emitted by walrus unconditionally zeroes every semaphore and
    runs its own all-engine barrier, and bass's preamble re-clears the
    kernel sem range + DMA queues at the start of every execution."""
    import concourse.tile as tile

    if getattr(tile.TileContext, "_cheap_teardown", False):
        return

    def _drain_and_barrier(self, tick_clock, wait_clock):
        popped = self.nc._tile_sem_poison_stack.pop()
        assert popped is self._sem_poison

    tile.TileContext._drain_and_barrier = _drain_and_barrier
    tile.TileContext._cheap_teardown = True


def _make_bass_no_entry_barrier():
    """Construct Bass() with its construction-time all_engine_barrier
    suppressed: the barrier only guards the preamble const memsets, which
    this kernel never reads (every cross-engine dependency in the body is
    semaphore-tracked by Tile).  Removing it lets the input DMA issue
    ~0.9us earlier."""
    import concourse.bass as bass

    orig = bass.Bass.all_engine_barrier
    bass.Bass.all_engine_barrier = lambda self, *a, **k: None
    try:
        nc = bass.Bass()
    finally:
        bass.Bass.all_engine_barrier = orig
    return nc


def build_bass():
    """Build the (uniform) single-core Bass program."""
    _ensure_path()
    import concourse.tile as tile
    from concourse import mybir
    from concourse.masks import make_identity

    _install_cheap_tile_teardown()

    f32 = mybir.dt.float32
    bf16 = mybir.dt.bfloat16
    SUB = mybir.AluOpType.subtract

    nc = _make_bass_no_entry_barrier()
    x = nc.dram_tensor("x", [128, 96], f32, kind="ExternalInput")
    acc_out = nc.dram_tensor("acc", [128, J_LOC], f32, kind="ExternalOutput")

    with tile.TileContext(nc) as tc:
        with (
            tc.tile_pool(name="sb", bufs=1) as sb,
            tc.tile_pool(name="ps", bufs=2, space="PSUM") as ps,
        ):
            # Input load first: two HWDGE queues (sync + scalar).  The DMA
            # fixed path (seq 565 + DGE 650 + sem-prop 900) dominates, so
            # issue as early as possible; with the entry barrier gone this
            # lands ~0.9us sooner.
            X = sb.tile([128, 96], f32)
            nc.sync.dma_start(out=X[0:64, :], in_=x[0:64, :])
            nc.scalar.dma_start(out=X[64:128, :], in_=x[64:128, :])

            # No-dependency constants, all emitted up front so they run
            # during the DMA wait.
            bias0 = sb.tile([128, 1], f32)
            nc.vector.memset(bias0[:, :], 0.0)
            # Warm the ACT sqrt table set early (table load ~1.3us).
            warm = sb.tile([1, 1], f32)
            nc.vector.memset(warm, 0.0)
            nc.scalar.activation(
                warm, warm, mybir.ActivationFunctionType.Sqrt,
                bias=bias0[:1, :], scale=1.0,
            )
            # W staging tiles [128 (i), (ci, kl, r)] with r padded 8->32 so
            # transposed rows land at partition base 32*kl.  The -1 memset
            # covers L rows 5,6 and R rows 3,4,7.
            WL = sb.tile([128, 512], bf16)
            WR = sb.tile([128, 512], bf16)
            nc.vector.memset(WL[:, :], -1.0)
            nc.vector.memset(WR[:, :], -1.0)
            EPSN = sb.tile([128, 16], bf16)
            nc.vector.memset(EPSN[:, :], -EPS)
            WLv = WL.rearrange("p (ci kl r) -> p ci kl r", ci=4, r=32)
            WRv = WR.rearrange("p (ci kl r) -> p ci kl r", ci=4, r=32)
            EPSNv = EPSN.rearrange("p (ci kl j) -> p ci kl j", ci=4, j=1)
            nc.vector.tensor_copy(WLv[:, :, :, 7:8], EPSNv[:, :, :, :])

            # Triangular merge mask (1 on p<=q, 0 below) replicated per ci,
            # and the PE-transpose identity: gpsimd, off the critical path.
            ident = sb.tile([128, 128], bf16)
            make_identity(nc, ident[:, :])
            TRI = sb.tile([128, 512], mybir.dt.int32)
            nc.gpsimd.memset(TRI[:, :], 1)
            nc.gpsimd.affine_select(
                out=TRI[:, :], in_=TRI[:, :],
                compare_op=mybir.AluOpType.is_ge,
                fill=0, base=0,
                # keep where -p + q >= 0  (upper triangle incl. diagonal)
                pattern=[[0, 4], [1, 128]],
                channel_multiplier=-1,
            )

            # ---- data-dependent prep (starts when the DMA lands) ----
            DB = sb.tile([128, 48], bf16)  # bf16(d)
            nc.vector.tensor_tensor(out=DB[:, :], in0=X[:, 0:48], in1=X[:, 48:96], op=SUB)
            DBv = DB.rearrange("p (ci kl c) -> p ci kl c", ci=4, c=3)
            # d columns of the stacks: R gets d (gpsimd), L gets -2d (ACT).
            nc.gpsimd.tensor_copy(WRv[:, :, :, 0:3], DBv[:, :, :, :])
            nc.scalar.mul(WLv[:, :, :, 0:3], DBv[:, :, :, :], -2.0)

            SQ = sb.tile([128, 48], f32)  # exact fp32 products of bf16 d
            nc.vector.tensor_mul(SQ[:, :], DB[:, :], DB[:, :])
            NN = sb.tile([128, 16], f32)  # m = -n
            nc.vector.tensor_reduce(
                out=NN[:, :],
                in_=SQ.rearrange("p (k c) -> p k c", c=3),
                axis=mybir.AxisListType.X,
                op=mybir.AluOpType.add,
                negate=True,
            )
            # 2-way bf16 split of m, packed [m1 | m2] in one tile.
            M12 = sb.tile([128, 32], bf16)
            M1 = M12[:, 0:16]
            M2 = M12[:, 16:32]
            nc.vector.tensor_copy(M1, NN[:, :])
            nc.vector.tensor_tensor(out=M2, in0=NN[:, :], in1=M1, op=SUB)
            M12v = M12.rearrange("p (j ci kl) -> p ci kl j", ci=4, kl=4)
            nc.vector.tensor_copy(WLv[:, :, :, 3:5], M12v[:, :, :, :])
            nc.scalar.copy(WRv[:, :, :, 5:7], M12v[:, :, :, :])

            # PE transposes: W column blocks -> stack rows at partitions
            # 32*kl + r.  PSR first (SR copy unblocks sooner).
            PSR = ps.tile([128, 512], bf16, tag="tr")
            PSL = ps.tile([128, 512], bf16, tag="tr")
            for ci in range(4):
                nc.tensor.transpose(
                    PSR[:, 128 * ci:128 * ci + 128], WR[:, 128 * ci:128 * ci + 128], ident[:, :]
                )
            for ci in range(4):
                nc.tensor.transpose(
                    PSL[:, 128 * ci:128 * ci + 128], WL[:, 128 * ci:128 * ci + 128], ident[:, :]
                )
            # PSUM -> SBUF on two engines in parallel.
            SR = sb.tile([128, 512], bf16)
            SL = sb.tile([128, 512], bf16)
            nc.vector.tensor_copy(SR[:, :], PSR[:, :])
            nc.scalar.copy(SL[:, :], PSL[:, :])

            # Per joint: G[128, 1536] f32 = 3 PSUM banks.
            #   [0,384)    off-diag ci0 (j in 128..512)
            #   [384,512)  off-diag ci2 (j in 384..512)
            #   [512,768)  off-diag ci1 (j in 256..512)
            #   [768,1280) diagonal blocks d0..d3 (full scale)
            # ACT reads [0,768) for even joints; for odd joints the even
            # joint's diagonal slab is first merged into this one's
            # (upper triangle from even, strict lower from odd) and ACT
            # reads [0,1280).
            ACC = sb.tile([128, J_LOC], f32)
            G_tiles = []
            for kl in range(J_LOC):
                G = ps.tile([128, 1536], f32, tag="g")
                G_tiles.append(G)
                r0 = 32 * kl
                rows = slice(r0, r0 + 8)
                # off-diagonal first: ACT's slab unblocks after 3 matmuls
                nc.tensor.matmul(
                    G[:, 0:384], lhsT=SL[rows, 0:128], rhs=SR[rows, 128:512],
                    start=True, stop=True, tile_position=(r0, 0),
                )
                nc.tensor.matmul(
                    G[:, 384:512], lhsT=SL[rows, 256:384], rhs=SR[rows, 384:512],
                    start=True, stop=True, tile_position=(r0, 0),
                )
                nc.tensor.matmul(
                    G[:, 512:768], lhsT=SL[rows, 128:256], rhs=SR[rows, 256:512],
                    start=True, stop=True, tile_position=(r0, 0),
                )
                for ci in range(4):
                    nc.tensor.matmul(
                        G[:, 768 + 128 * ci:768 + 128 * ci + 128],
                        lhsT=SL[rows, 128 * ci:128 * ci + 128],
                        rhs=SR[rows, 128 * ci:128 * ci + 128],
                        start=True, stop=True, tile_position=(r0, 0),
                    )
                if kl % 2 == 0:
                    # even joint: sqrt+sum the off-diag slab only
                    nc.scalar.activation(
                        out=G[:, 0:768], in_=G[:, 0:768],
                        func=mybir.ActivationFunctionType.Sqrt,
                        bias=bias0[:, :], scale=1.0,
                        accum_out=ACC[:, kl:kl + 1],
                    )
                else:
                    # merge even joint's diag slab into this one's, then
                    # sqrt+sum off-diag + merged diag in one pass
                    Gprev = G_tiles[kl - 1]
                    nc.vector.copy_predicated(
                        out=G[:, 768:1280], mask=TRI[:, :], data=Gprev[:, 768:1280],
                    )
                    nc.scalar.activation(
                        out=G[:, 0:1280], in_=G[:, 0:1280],
                        func=mybir.ActivationFunctionType.Sqrt,
                        bias=bias0[:, :], scale=1.0,
                        accum_out=ACC[:, kl:kl + 1],
                    )
            nc.sync.dma_start(out=acc_out[:, :], in_=ACC[:, :])

    return nc


def make_in_maps(pred_3d: np.ndarray, reconstructed_3d: np.ndarray):
    """Shard: core c gets joints [4c, 4c+4), packed as [128, 96] f32 with
    x[p, t*48 + ci*12 + kl*3 + c] = (pred,recon)[128*ci + p, 4*cc + kl, c]."""
    pred = np.asarray(pred_3d, dtype=np.float32)
    recon = np.asarray(reconstructed_3d, dtype=np.float32)
    in_maps = []
    for cc in range(NCORES):
        sl = slice(J_LOC * cc, J_LOC * cc + J_LOC)
        arr = np.stack([pred[:, sl, :], recon[:, sl, :]])  # [2, 512, 4, 3]
        arr = (
            arr.reshape(2, 4, 128, J_LOC * 3)
            .transpose(2, 0, 1, 3)
            .reshape(128, 96)
        )
        in_maps.append({"x": np.ascontiguousarray(arr)})
    return in_maps


def _get_nc():
    if "nc" not in _STATE:
        _STATE["nc"] = build_bass()
    return _STATE["nc"]


def reduce_outputs(results) -> np.ndarray:
    total = np.float64(0.0)
    for r in results:
        total += np.asarray(r["acc"], dtype=np.float64).sum()
    # every unordered pair is counted exactly once at full scale
    loss = total / (J * NUM_PAIRS)
    return np.float32(loss)


def kernel(pred_3d: np.ndarray, reconstructed_3d: np.ndarray) -> np.ndarray:
    _ensure_path()
    install_walrus_wait_split()
    from concourse.bass_utils import run_bass_kernel_spmd

    nc = _get_nc()
    in_maps = make_in_maps(pred_3d, reconstructed_3d)
    res = run_bass_kernel_spmd(nc, in_maps, list(range(NCORES)))
    return reduce_outputs(res.results)
